# revision 4
# baseline (speedup 1.0000x reference)
"""Single-head attention (B=8, T=4096, E=768, H=64) on 8 TRN2 NeuronCores.

Sharding: data-parallel over batch B - one batch element per core, Q/K/V
projection weights replicated. Per core:

  phase 1: SWDGE cast-loads x as bf16; PE-transpose 128x128 blocks -> xT
           [E,T] in SBUF (bf16 rate, 4 blocks batched per PSUM tile so DVE
           drains them in one copy)
  phase 2: packed projections: stationary [Wq|Wk] -> one matmul emits q on
           psum partitions 0:64 and k on 64:128; ACT drains q -> qT(lo) and
           k -> kT(hi). v separately -> vT. Per-chunk SBUF->SBUF DMAs then
           mirror qT to partitions 64:128 and kT to 0:64 so MM1 can run
           2-way row-packed (PE array rows 0:63 and 64:127 concurrently,
           tile_position (0,0)/(64,0) auto-derived from base partitions).
  phase 3: PE-transpose vT -> v tiles [128s, 64], append ones column
           -> [128, 65] (row 64 of MM2 output accumulates the softmax
           denominator)
  phase 4 (software-pipelined over 16 s-tile PAIRS x 8 q-groups):
           MM1 pair: S^T halves [128s, 512t] for s0 (rows 0:63) and s1
             (rows 64:127) land in one [128, 1024] 2-bank PSUM tile
           exp of the 1024-wide tile alternates between ACT (hardware Exp,
             free scale) and DVE (custom fused op: monic-cubic
             (v+a)((v+b)^2+c) followed by 3 squarings = exp(8w) with the
             variable pre-scaled by lambda/8 into qT; 8 ALU stages, one
             DVE instruction at 1 elem/cycle/lane) - the two engines halve
             the exp bottleneck that dominates an ACT-only kernel
           MM2 x2: out^T [65, 512t] += [v|1]_s.T @ exp (PSUM accumulate)
           tail per group: PE-transpose out^T blocks, multiply by
             reciprocal of the denominator row, DMA [t, h] to DRAM.

  All matmul-facing tensors are bf16 (rel err ~2.6e-3 vs fp32 reference;
  the DVE exp path adds ~1.3e-3 on its tiles; gate is 2e-2).
"""

import math
import os
import sys

for _p in ("/opt/trn_rl_repo", "/root/.axon_site/_ro/trn_rl_repo"):
    if os.path.isdir(_p) and _p not in sys.path:
        sys.path.insert(0, _p)

import numpy as np

import concourse.bacc as bacc
import concourse.tile as tile
from concourse import mybir
from concourse.bass_utils import run_bass_kernel_spmd
from concourse.masks import make_identity

B, T, E, H = 8, 4096, 768, 64
P = 128
NE = E // P            # 6 e-chunks
NT = T // P            # 32 s-tiles
GQ = 512               # q-group width (t rows)
NG = T // GQ           # 8 q-groups
NPAIR = NT // 2        # 16 s-tile pairs per q-group
SCALE = float(H) ** -0.5

F32 = mybir.dt.float32
BF16 = mybir.dt.bfloat16

# --- custom DVE exp: q(v) = (v+A)((v+B)^2 + C), out = q^8 ------------------
# Fit of e^w deg-3 (relative-error weighted) on |w| <= 0.36, monic form via
# v = LAM*w with LAM = cbrt(c3). stp holds v = logits*LAM/8 (folded into the
# qT prescale); ACT path recovers exp(logits) via its free scale 8/LAM.
EXP_A = 0.890217935821643
EXP_B = 0.3930562704875204
EXP_C = 0.9687638651114505
LAM = 0.5486231552172741

_EXP8_OP = None


def _register_exp8():
    global _EXP8_OP
    if _EXP8_OP is not None:
        return _EXP8_OP
    import concourse.dve_ops as dvo
    from concourse.dve_spec import C0, C1, C2, Spec, Src0, lower
    from concourse.dve_uop import DveOpSpec

    name = "EXP8_ATTN_ANT"
    for op in dvo.OPS:
        if op.name == name:
            _EXP8_OP = op
            return op

    t1 = Src0 + C0
    t2 = Src0 + C1
    t3 = t2 * t2
    t4 = t3 + C2
    q = t1 * t4
    r = q * q
    r2 = r * r
    body = r2 * r2

    def ref(in0, in1, s0, s1, imm2):
        x = in0.astype(np.float32)
        q = (x + s0) * ((x + s1) * (x + s1) + imm2)
        r = (q * q).astype(np.float32)
        r = (r * r).astype(np.float32)
        return (r * r).astype(np.float32)

    spec = Spec(body=body, reference=ref)
    row = dvo._CUSTOM_DVE_ROW_BASE + len(dvo.OPS)
    shas = {}
    for ver in ("v3", "v4"):
        s = DveOpSpec(name=name, opcode=row, uops=lower(spec, ver=ver),
                      rd1_en=False)
        shas[ver] = s.sha(ver)
    op = dvo.DveOp(name, spec, subdim=False, uops_sha=shas)
    dvo.OPS.append(op)
    dvo.CUSTOM_DVE_SPECS[name] = spec
    dvo._SUB_OPCODE_FOR_NAME[name] = row
    _EXP8_OP = op
    return op


def build_nc(reps=1, rep_scope="all"):
    nc = bacc.Bacc("TRN2", target_bir_lowering=False, debug=False, num_devices=8)

    x = nc.dram_tensor("x", [T, E], F32, kind="ExternalInput")
    wq = nc.dram_tensor("Wq", [E, H], F32, kind="ExternalInput")
    wk = nc.dram_tensor("Wk", [E, H], F32, kind="ExternalInput")
    wv = nc.dram_tensor("Wv", [E, H], F32, kind="ExternalInput")
    bq = nc.dram_tensor("bq", [H], F32, kind="ExternalInput")
    bk = nc.dram_tensor("bk", [H], F32, kind="ExternalInput")
    bv = nc.dram_tensor("bv", [H], F32, kind="ExternalInput")
    out = nc.dram_tensor("out", [T, H], F32, kind="ExternalOutput")

    with tile.TileContext(nc) as tc:
        with tc.tile_pool(name="consts", bufs=1) as consts:
            ident = consts.tile([P, P], F32)
            make_identity(nc, ident)
            identb = consts.tile([P, P], BF16, tag="identb")
            nc.vector.tensor_copy(identb, ident)

            # packed [Wq | Wk] stationary (bf16) + separate Wv
            wqk = consts.tile([P, NE, P], BF16, tag="wqk")
            wvt = consts.tile([P, NE, H], BF16, tag="wv")
            for cols, wdram, tag in ((slice(0, H), wq, "fq"),
                                     (slice(H, P), wk, "fk")):
                wtf = consts.tile([P, NE, H], F32, tag="wf" + tag)
                nc.sync.dma_start(
                    out=wtf, in_=wdram[:, :].rearrange("(c p) h -> p c h", p=P)
                )
                nc.vector.tensor_copy(wqk[:, :, cols], wtf)
            wvf = consts.tile([P, NE, H], F32, tag="wfv")
            nc.sync.dma_start(
                out=wvf, in_=wv[:, :].rearrange("(c p) h -> p c h", p=P)
            )
            nc.vector.tensor_copy(wvt, wvf)

            # biases: q at partitions 0:64 (pre-scaled), k at 64:128, v at 0:64
            bq_t = consts.tile([H, 1], F32, tag="bq")
            nc.sync.dma_start(out=bq_t, in_=bq[:].rearrange("(h o) -> h o", o=1))
            bqs = consts.tile([H, 1], F32, tag="bqs")
            nc.scalar.mul(out=bqs, in_=bq_t, mul=SCALE * LAM / 8.0)
            bk_t = consts.tile([P, 1], F32, tag="bk")
            nc.sync.dma_start(
                out=bk_t[H:P, :], in_=bk[:].rearrange("(h o) -> h o", o=1)
            )
            bv_t = consts.tile([H, 1], F32, tag="bv")
            nc.sync.dma_start(out=bv_t, in_=bv[:].rearrange("(h o) -> h o", o=1))

            with tc.tile_pool(name="persist", bufs=1) as persist:
                qTf = persist.tile([P, T], BF16, tag="qTf")
                kTf = persist.tile([P, T], BF16, tag="kTf")
                vT = persist.tile([H, T], BF16, tag="vT")
                v1 = persist.tile([P, NT, H + 1], BF16, tag="v1")

                setup_reps = reps if rep_scope in ("all", "setup") else 1
                attn_reps = 1 if rep_scope == "setup" else reps
                for _ in range(setup_reps):
                    _setup(nc, tc, x, wqk, wvt, bqs, bk_t, bv_t, identb,
                           qTf, kTf, vT, v1)
                for _ in range(attn_reps):
                    _attention(nc, tc, out, ident, qTf, kTf, v1)
    nc.compile()
    return nc


def _setup(nc, tc, x, wqk, wvt, bqs, bk_t, bv_t, identb, qTf, kTf, vT, v1):
    IDENT = mybir.ActivationFunctionType.Identity
    # ---------------- phase 1: load x (bf16 cast) and transpose to xT ------
    with (
        tc.tile_pool(name="xT_pool", bufs=1) as xT_pool,
        tc.tile_pool(name="xin", bufs=3) as xin,
        tc.tile_pool(name="ps_t", bufs=4, space="PSUM") as ps_t,
        tc.tile_pool(name="ps_p", bufs=2, space="PSUM") as ps_p,
    ):
        xT = xT_pool.tile([P, NE, T], BF16, tag="xT")
        SUB = int(os.environ.get("KERNEL_CAST_SUB", "8"))
        for k in range(NT // SUB):
            xt = xin.tile([P, SUB, E], BF16, tag="x")
            nc.gpsimd.dma_start(
                out=xt,
                in_=x[k * SUB * P:(k + 1) * SUB * P, :].rearrange(
                    "(i p) e -> p i e", p=P),
            )
            for i_sub in range(SUB):
                i = k * SUB + i_sub
                for c4 in range(NE // 4 + (1 if NE % 4 else 0)):
                    cs = list(range(c4 * 4, min(NE, (c4 + 1) * 4)))
                    pst = ps_t.tile([P, 4, P], BF16, tag="t")
                    for ci, c in enumerate(cs):
                        nc.tensor.transpose(
                            pst[:, ci, :],
                            xt[:, i_sub, c * P:(c + 1) * P], identb)
                    nc.vector.tensor_copy(
                        xT[:, cs[0]:cs[-1] + 1, i * P:(i + 1) * P],
                        pst[:, 0:len(cs), :])

        # ---------------- phase 2: projections -> qT/kT/vT + mirrors -------
        for j in range(T // 512):
            jsl = slice(j * 512, (j + 1) * 512)
            psqk = ps_p.tile([P, 512], F32, tag="pqk")
            for c in range(NE):
                nc.tensor.matmul(
                    psqk, wqk[:, c, :], xT[:, c, jsl],
                    start=(c == 0), stop=(c == NE - 1),
                )
            nc.scalar.activation(
                out=qTf[0:H, jsl], in_=psqk[0:H, :], func=IDENT,
                bias=bqs, scale=SCALE * LAM / 8.0,
            )
            nc.scalar.activation(
                out=kTf[H:P, jsl], in_=psqk[H:P, :], func=IDENT,
                bias=bk_t[H:P, :], scale=1.0,
            )
            psv = ps_p.tile([H, 512], F32, tag="pv")
            for c in range(NE):
                nc.tensor.matmul(
                    psv, wvt[:, c, :], xT[:, c, jsl],
                    start=(c == 0), stop=(c == NE - 1),
                )
            nc.scalar.activation(
                out=vT[:, jsl], in_=psv, func=IDENT, bias=bv_t, scale=1.0,
            )
            # mirror q to partitions 64:128 and k to 0:64 for row-packed MM1
            nc.sync.dma_start(out=qTf[H:P, jsl], in_=qTf[0:H, jsl])
            nc.sync.dma_start(out=kTf[0:H, jsl], in_=kTf[H:P, jsl])

    # ---------------- phase 3: vT -> v1 tiles [128, 65] --------------------
    with tc.tile_pool(name="ps_v", bufs=2, space="PSUM") as ps_v:
        nc.vector.memset(v1[:, :, H:H + 1], 1.0)
        idv = identb[0:H, 0:H]
        for s in range(NT):
            psv = ps_v.tile([P, H], BF16, tag="v")
            nc.tensor.transpose(psv, vT[:, s * P:(s + 1) * P], idv)
            nc.vector.tensor_copy(v1[:, s, 0:H], psv)


def _attention(nc, tc, out, ident, qTf, kTf, v1):
    EXPF = mybir.ActivationFunctionType.Exp
    exp8 = _register_exp8()
    nopack = os.environ.get("KERNEL_NOPACK") == "1"
    dve_mod = int(os.environ.get("KERNEL_DVE_MOD", "2"))  # 1/dve_mod to DVE
    lookahead = int(os.environ.get("KERNEL_LOOKAHEAD", "1"))
    with (
        tc.tile_pool(name="ps_st", bufs=3, space="PSUM") as ps_st,
        tc.tile_pool(name="ps_out", bufs=2, space="PSUM") as ps_out,
        tc.tile_pool(name="expp", bufs=6) as expp,
        tc.tile_pool(name="outsb", bufs=2) as outsb,
        tc.tile_pool(name="stage", bufs=2) as stage,
        tc.tile_pool(name="recp", bufs=4) as recp,
    ):
        def mm1(g, p):
            gsl = slice(g * GQ, (g + 1) * GQ)
            s0, s1 = 2 * p, 2 * p + 1
            stp = ps_st.tile([P, 2, 512], F32, tag="st")
            nc.tensor.matmul(
                stp[:, 0, :], kTf[0:H, s0 * P:(s0 + 1) * P], qTf[0:H, gsl],
                start=True, stop=True,
            )
            if nopack:
                nc.tensor.matmul(
                    stp[:, 1, :], kTf[0:H, s1 * P:(s1 + 1) * P], qTf[0:H, gsl],
                    start=True, stop=True,
                )
            else:
                nc.tensor.matmul(
                    stp[:, 1, :], kTf[H:P, s1 * P:(s1 + 1) * P], qTf[H:P, gsl],
                    start=True, stop=True,
                )
            return stp

        outps = {}
        it = [(g, p) for g in range(NG) for p in range(NPAIR)]
        stps = [mm1(*it[i]) for i in range(lookahead)]
        for idx, (g, p) in enumerate(it):
            if p == 0:
                outps[g] = ps_out.tile([H + 1, GQ], F32, tag="o",
                                       name=f"outp{g}")
            stp = stps.pop(0) if lookahead else mm1(g, p)
            ex = expp.tile([P, 2, 512], BF16, tag="ex")
            if idx % dve_mod == 0:
                nc.vector._custom_dve(
                    exp8, out=ex, in0=stp,
                    s0=EXP_A, s1=EXP_B, imm2=EXP_C,
                )
            else:
                nc.scalar.activation(out=ex, in_=stp, func=EXPF,
                                     scale=8.0 / LAM)
            if lookahead and idx + lookahead < len(it):
                stps.append(mm1(*it[idx + lookahead]))
            s0, s1 = 2 * p, 2 * p + 1
            nc.tensor.matmul(
                outps[g], v1[:, s0, :], ex[:, 0, :],
                start=(p == 0), stop=False,
            )
            nc.tensor.matmul(
                outps[g], v1[:, s1, :], ex[:, 1, :],
                start=False, stop=(p == NPAIR - 1),
            )
            if p == NPAIR - 1:
                osb = outsb.tile([H + 1, GQ], F32, tag="osb", name=f"osb{g}")
                nc.scalar.copy(osb, outps.pop(g))
                _attn_tail(nc, out, ident, osb, stage, recp, ps_st, g)


def _attn_tail(nc, out, ident, osb, stage, recp, ps_st, g):
    nb = GQ // P
    st_t = stage.tile([P, nb, H], F32, tag="stage", name=f"st_t{g}")
    for b in range(nb):
        pst = ps_st.tile([P, H + 1], F32, tag="st")
        nc.tensor.transpose(
            pst, osb[:, b * P:(b + 1) * P], ident[0:H + 1, 0:H + 1]
        )
        rec = recp.tile([P, 1], F32, tag="rec")
        nc.vector.reciprocal(rec, pst[:, H:H + 1])
        nc.vector.tensor_scalar_mul(st_t[:, b, :], pst[:, 0:H], rec)
    nc.sync.dma_start(
        out=out[g * GQ:(g + 1) * GQ, :].rearrange("(b p) h -> p b h", p=P),
        in_=st_t,
    )


_NC_CACHE = {}


def _get_nc():
    if "nc" not in _NC_CACHE:
        _NC_CACHE["nc"] = build_nc()
    return _NC_CACHE["nc"]


def kernel(x, Wq, bq, Wk, bk, Wv, bv):
    x = np.ascontiguousarray(np.asarray(x, dtype=np.float32))
    in_common = {
        "Wq": np.ascontiguousarray(np.asarray(Wq, np.float32)),
        "Wk": np.ascontiguousarray(np.asarray(Wk, np.float32)),
        "Wv": np.ascontiguousarray(np.asarray(Wv, np.float32)),
        "bq": np.ascontiguousarray(np.asarray(bq, np.float32)),
        "bk": np.ascontiguousarray(np.asarray(bk, np.float32)),
        "bv": np.ascontiguousarray(np.asarray(bv, np.float32)),
    }
    nc = _get_nc()
    in_maps = [dict(in_common, x=x[b]) for b in range(B)]
    res = run_bass_kernel_spmd(nc, in_maps, core_ids=list(range(B)))
    return np.stack([res.results[b]["out"] for b in range(B)], axis=0)


if __name__ == "__main__":
    rng = np.random.default_rng(0)
    xs = rng.standard_normal((B, T, E), dtype=np.float32)
    s = 1.0 / np.sqrt(E)
    mk = lambda *shape: rng.uniform(-s, s, size=shape).astype(np.float32)
    o = kernel(xs, mk(E, H), mk(H), mk(E, H), mk(H), mk(E, H), mk(H))
    print("out", o.shape, o.dtype, float(np.abs(o).max()))
